# revision 30
# baseline (speedup 1.0000x reference)
"""Trainium2 Bass kernel for nn_CausalTransformerEncoder.

Sharding: 8 cores = 2 (batch) x 4 (sequence chunks of 512 tokens).
Per layer: each core computes LN + QKV for its own 512 tokens, AllGathers
K^T/V within its 4-core batch group, runs attention over the full key
range (block-causal handled by per-core bias tables + a local diagonal
pass with static triangular masks), then out-proj, LN2 and FFN on its own
tokens. Residual stream stays fp32 in SBUF; q/k/scores run fp8 DoubleRow
(validated numerically), everything else bf16 (fp8 there busts the 2e-2
error gate).

Softmax: exp without max subtraction (scores are small; validated on host),
denominator for free via a ones-column appended to V (matmul M=65 outputs
o rows 0:64 and the sum at row 64). Key-padding from `lengths` is handled
by zeroing V rows + ones-column at the source, so masked keys drop out of
both numerator and denominator with no extra masking work.

LayerNorm stats (sum / sum-of-squares) are computed for free on the
residual adds (scalar_tensor_tensor accum_out + an overlapped Square
activation per tile), so the LN serial chain no longer idles the PE.
"""

import sys
import os

for _p in ("/opt/trn_rl_repo", os.path.expanduser("~/.axon_site/_ro/trn_rl_repo")):
    if os.path.isdir(_p) and _p not in sys.path:
        sys.path.insert(0, _p)

import numpy as np
import ml_dtypes

import concourse.bass as bass
from concourse import bacc
import concourse.mybir as mybir
import concourse.tile as tile
from concourse.bass import ts
from concourse.bass_utils import run_bass_kernel_spmd

F32 = mybir.dt.float32
BF16 = mybir.dt.bfloat16
F8 = mybir.dt.float8e4
AF = mybir.ActivationFunctionType
AX = mybir.AxisListType
DR = mybir.MatmulPerfMode.DoubleRow
MUL = mybir.AluOpType.mult
ADD = mybir.AluOpType.add

B, T, D, H, L, FF = 2, 2048, 1024, 16, 4, 4096
DH = D // H          # 64
P = 128
CH = 512             # tokens per core
NT = CH // P         # 4 token tiles per core
NG = D // P          # 8 feature sub-tiles
NF = FF // P         # 32 ffn sub-tiles
NHP = H // 2         # 8 head pairs
NCHUNK = 4           # sequence chunks per batch group
NDC = NCHUNK - 1     # dense chunks actually attended (chunk 3 is always
                     # either masked or covered by the local diagonal pass)
SCALE = 1.0 / np.sqrt(DH)
NEG = -1.0e9
WS = 1024.0          # fp8 q/k weight pre-scale (descaled at the psum copy)
KB = 4 * 2 * CH      # fp8 K bytes per partition in the collective payload
VB = NT * H * DH * 2  # bf16 V bytes per partition (shipped bitcast as fp8)

_CACHE = {}


def _build(with_bias: bool, nlayers: int = L, dump: bool = False, for_sim: bool = False):
    nc = bacc.Bacc("TRN2", target_bir_lowering=False, debug=False, num_devices=8)

    # ---------------- I/O ----------------
    x0 = nc.dram_tensor("x0", [CH, D], F32, kind="ExternalInput")
    wq = nc.dram_tensor("wq", [L, D, D], F8, kind="ExternalInput")
    wk = nc.dram_tensor("wk", [L, D, D], F8, kind="ExternalInput")
    wv = nc.dram_tensor("wv", [L, D, D], BF16, kind="ExternalInput")
    wo = nc.dram_tensor("wo", [L, D, D], BF16, kind="ExternalInput")
    w1 = nc.dram_tensor("w1", [L, D, FF], BF16, kind="ExternalInput")
    w2 = nc.dram_tensor("w2", [L, FF, D], BF16, kind="ExternalInput")
    b1c = nc.dram_tensor("b1c", [L, P, NF], F32, kind="ExternalInput")
    mdense = nc.dram_tensor("mdense", [P, NCHUNK], F32, kind="ExternalInput")
    padcol = nc.dram_tensor("padcol", [P, NT], F32, kind="ExternalInput")
    padfull = nc.dram_tensor("padfull", [P, NCHUNK, NT], F32, kind="ExternalInput")
    trim = nc.dram_tensor("trim", [NT, P, CH], BF16, kind="ExternalInput")
    ident = nc.dram_tensor("ident", [P, P], BF16, kind="ExternalInput")
    if with_bias:
        brows = nc.dram_tensor("brows", [3, D], BF16, kind="ExternalInput")
    y = nc.dram_tensor("y", [CH, D], F32, kind="ExternalOutput")

    # per-layer DRAM for the collectives, split into head-halves so the
    # first half's dense attention starts while the second is in flight:
    # fp8 K (packed, 2 f-slots) + bf16 V (8 heads, bitcast)
    HKB = KB // 2
    HVB = VB // 2
    kvin = [
        [nc.dram_tensor(f"kvin{l}_{h}", [P, HKB + HVB], F8) for h in range(2)]
        for l in range(L)
    ]
    kvall = [
        [
            nc.dram_tensor(f"kvall{l}_{h}", [NCHUNK, P, HKB + HVB], F8)
            for h in range(2)
        ]
        for l in range(L)
    ]

    # ---------------- persistent SBUF ----------------
    xs = nc.alloc_sbuf_tensor("xs", [P, NT, D], F32).ap()
    hT = nc.alloc_sbuf_tensor("hT", [P, NG, CH], BF16).ap()
    hT8 = nc.alloc_sbuf_tensor("hT8", [P, NG, CH], F8).ap()
    # packed q/k for fp8 DoubleRow scores: head h lives at partitions
    # [32*(h%4), +32), free group h//4, with feature f at (f%32, slot f//32)
    qT8 = nc.alloc_sbuf_tensor("qT8", [P, 4, 2, CH], F8).ap()
    kTloc8 = nc.alloc_sbuf_tensor("kTloc8", [P, 4, 2, CH], F8).ap()
    oT = nc.alloc_sbuf_tensor("oT", [P, NHP, CH], BF16).ap()
    oTb = nc.alloc_sbuf_tensor("oTb", [64, NHP, CH], BF16).ap()
    RbA = nc.alloc_sbuf_tensor("RbA", [64, NHP, CH], BF16).ap()
    RbB = nc.alloc_sbuf_tensor("RbB", [64, NHP, CH], BF16).ap()
    kcache = nc.alloc_sbuf_tensor("kcache", [P, NDC, 4, 2, CH], F8).ap()
    vcache = nc.alloc_sbuf_tensor("vcache", [P, NDC, NT, H, DH + 1], BF16).ap()
    mdense_s = nc.alloc_sbuf_tensor("mdense_s", [P, NCHUNK], F32).ap()
    padcol_s = nc.alloc_sbuf_tensor("padcol_s", [P, NT], F32).ap()
    padfull_s = nc.alloc_sbuf_tensor("padfull_s", [P, NCHUNK, NT], F32).ap()
    b1c_s = nc.alloc_sbuf_tensor("b1c_s", [P, L, NF], F32).ap()
    trim_s = nc.alloc_sbuf_tensor("trim_s", [P, NT, CH], BF16).ap()
    ident_s = nc.alloc_sbuf_tensor("ident_s", [P, P], BF16).ap()
    eps_s = nc.alloc_sbuf_tensor("eps_s", [P, 1], F32).ap()
    stats = nc.alloc_sbuf_tensor("stats", [P, 6, NT], F32).ap()  # sum,sq,mu,mu2,var,rs
    stp = nc.alloc_sbuf_tensor("stp", [P, 4, NT], F32).ap()  # per-half partials
    if with_bias:
        bias_bc = nc.alloc_sbuf_tensor("bias_bc", [P, 3, D], BF16).ap()

    with tile.TileContext(nc) as tc:
        with (
            tc.tile_pool(name="big", bufs=4) as big,       # qT/kTloc/vloc/gT overlay
            tc.tile_pool(name="wA", bufs=3) as wA,         # [P, NG, 512] weights
            tc.tile_pool(name="wB", bufs=4) as wB,         # [P, 512] w2 tiles
            tc.tile_pool(name="htm", bufs=2) as htmp,      # token-major h staging
            tc.tile_pool(name="attn", bufs=4) as attnp,    # exp outputs
            tc.tile_pool(name="lrec", bufs=2) as lrecp,
            tc.tile_pool(name="lrecf", bufs=2) as lrecf,    # softmax denom recip
            tc.tile_pool(name="sc", bufs=2, space="PSUM") as scp,    # 2 banks/slot
            tc.tile_pool(name="oacc", bufs=2, space="PSUM") as oaccp,  # 1 bank
            tc.tile_pool(name="mm", bufs=2, space="PSUM") as mmp,      # 1 bank
        ):
            # one-time loads
            nc.vector.memset(eps_s, 1e-5)
            nc.sync.dma_start(xs, x0.rearrange("(tt p) d -> p tt d", p=P))
            nc.sync.dma_start(mdense_s, mdense[:])
            nc.sync.dma_start(padcol_s, padcol[:])
            nc.sync.dma_start(padfull_s, padfull[:])
            nc.sync.dma_start(b1c_s, b1c.rearrange("l p x -> p l x"))
            nc.sync.dma_start(trim_s, trim.rearrange("j p t -> p j t"))
            nc.sync.dma_start(ident_s, ident[:])

            ssum = stats[:, 0]
            ssq = stats[:, 1]

            def ln_to_hT(lix, stats_ready):
                """LayerNorm(xs) -> token-major bf16 -> transpose into hT.

                When stats_ready, ssum/ssq were already filled by the
                preceding residual adds (accum_out) + Square activations.
                """
                mu = stats[:, 2]
                mu2 = stats[:, 3]
                var = stats[:, 4]
                rs = stats[:, 5]
                if not stats_ready:
                    for tt in range(NT):
                        nc.vector.reduce_sum(
                            out=ssum[:, tt : tt + 1], in_=xs[:, tt], axis=AX.X
                        )
                        scr = htmp.tile([P, D], BF16, tag="htm")
                        nc.scalar.activation(
                            scr, xs[:, tt], AF.Square, accum_out=ssq[:, tt : tt + 1]
                        )
                nc.vector.tensor_scalar_mul(mu, ssum, 1.0 / D)
                nc.vector.tensor_mul(out=mu2, in0=mu, in1=mu)
                nc.vector.tensor_scalar_mul(var, ssq, 1.0 / D)
                nc.vector.tensor_sub(out=var, in0=var, in1=mu2)
                # rs = 1/sqrt(var + eps)
                nc.scalar.activation(var, var, AF.Sqrt, bias=eps_s[:, 0:1])
                nc.vector.reciprocal_approx_fast(out=rs, in_=var)
                for tt in range(NT):
                    htile = htmp.tile([P, D], BF16, tag="htm")
                    nc.vector.tensor_scalar(
                        htile,
                        xs[:, tt],
                        mu[:, tt : tt + 1],
                        rs[:, tt : tt + 1],
                        mybir.AluOpType.subtract,
                        mybir.AluOpType.mult,
                    )
                    for g in range(NG):
                        pt = mmp.tile([P, P], BF16, tag="mm")
                        nc.tensor.transpose(pt, htile[:, ts(g, P)], ident_s)
                        nc.vector.tensor_copy(out=hT[:, g, ts(tt, P)], in_=pt)
                # fp8 copy for the DoubleRow q/k (and ffn rhs) matmuls,
                # split per g-pair so consumers can start early
                with nc.allow_low_precision(reason="fp8 copy for q/k matmuls"):
                    for kp in range(NG // 2):
                        nc.vector.tensor_copy(
                            out=hT8[:, 2 * kp : 2 * kp + 2],
                            in_=hT[:, 2 * kp : 2 * kp + 2],
                        )

            def qk_one(l, wmat, dst, eng, gs=range(NG)):
                # Q^T or K^T via full-width fp8 DoubleRow matmuls (the ISA
                # rejects DoubleRow with a column-sliced psum dst), then a
                # descaled fp8 copy (on ScalarE; DVE is loaded here) and 4
                # partition-shift DMAs per g-block into the packed layout.
                for g in gs:
                    wt = wA.tile([P, NG, P], F8, tag="wA")
                    nc.sync.dma_start(
                        wt,
                        wmat[l, :, ts(g, P)].rearrange(
                            "(kd p) f -> p kd f", p=P
                        ),
                    )
                    ps = mmp.tile([P, CH], F32, tag="mm")
                    for kp in range(NG // 2):
                        nc.tensor.matmul(
                            ps,
                            lhsT=wt[:, 2 * kp : 2 * kp + 2],
                            rhs=hT8[:, 2 * kp : 2 * kp + 2],
                            start=(kp == 0),
                            stop=(kp == NG // 2 - 1),
                            perf_mode=DR,
                        )
                    st8 = wB.tile([P, CH], F8, tag="wB")
                    nc.scalar.activation(st8, ps, AF.Copy, scale=1.0 / WS)
                    # feature f of head h -> (partition 32*(h%4) + f//2,
                    # slot f%2): the DMA's partition-major flattening folds
                    # [64, 512] into [32, 2, 512] exactly, one DMA per head
                    for e in range(2):
                        head = 2 * g + e
                        eng.dma_start(
                            dst[32 * (head % 4) : 32 * (head % 4) + 32,
                                head // 4],
                            st8[64 * e : 64 * e + 64],
                        )

            def v_part(l, vloc, ns=range(2)):
                # V (token-major, padded rows zeroed, into aug layout)
                for n in ns:
                    wvt = wA.tile([P, NG, CH], BF16, tag="wA")
                    nc.sync.dma_start(
                        wvt,
                        wv[l, :, ts(n, CH)].rearrange("(kd p) f -> p kd f", p=P),
                    )
                    for tt in range(NT):
                        ps = mmp.tile([P, CH], F32, tag="mm")
                        for kd in range(NG):
                            nc.tensor.matmul(
                                ps,
                                lhsT=hT[:, kd, ts(tt, P)],
                                rhs=wvt[:, kd],
                                start=(kd == 0),
                                stop=(kd == NG - 1),
                            )
                        if with_bias:
                            nc.vector.tensor_tensor(
                                ps, ps, bias_bc[:, 0, ts(n, CH)], mybir.AluOpType.add
                            )
                        nc.vector.tensor_scalar_mul(
                            vloc[:, tt, 8 * n : 8 * (n + 1), 0:DH],
                            ps.rearrange("p (h e) -> p h e", h=8),
                            padcol_s[:, tt : tt + 1],
                        )
                    # ones column = padcol (zero for invalid keys)
                    nc.vector.tensor_copy(
                        out=vloc[:, :, 8 * n : 8 * (n + 1), DH : DH + 1],
                        in_=padcol_s[:, :, None, None].to_broadcast(
                            [P, NT, H // 2, 1]
                        ),
                    )

            def allgather(l, vloc, hs=range(2)):
                # two half-collectives (heads 0-7, then 8-15); kvin writes
                # ride the ScalarE DGE ring and cache fills the GpSimd ring
                # so their data/AG-semaphore waits never block the weight
                # loads flowing on the sync ring
                for h in hs:
                    nc.scalar.dma_start(
                        kvin[l][h][:, 0:HKB].rearrange(
                            "p (f j t) -> p f j t", f=2, j=2
                        ),
                        kTloc8[:, 2 * h : 2 * h + 2],
                    )
                    for tt in range(NT):
                        nc.scalar.dma_start(
                            kvin[l][h][
                                :, HKB + tt * H * DH : HKB + (tt + 1) * H * DH
                            ].bitcast(BF16).rearrange("p (h e) -> p h e", h=H // 2),
                            vloc[:, tt, 8 * h : 8 * h + 8, 0:DH],
                        )
                    if for_sim:
                        nc.scalar.dma_start(kvall[l][h][0], kvin[l][h][:])
                    else:
                        nc.gpsimd.collective_compute(
                            "AllGather",
                            mybir.AluOpType.bypass,
                            replica_groups=[[0, 1, 2, 3], [4, 5, 6, 7]],
                            ins=[kvin[l][h][:]],
                            outs=[kvall[l][h][:]],
                        )
                    nc.gpsimd.dma_start(
                        kcache[:, :, 2 * h : 2 * h + 2],
                        kvall[l][h][0:NDC, :, 0:HKB].rearrange(
                            "c p (f j t) -> p c f j t", f=2, j=2
                        ),
                    )
                    for c in range(NDC):
                        for tt in range(NT):
                            nc.gpsimd.dma_start(
                                vcache[:, c, tt, 8 * h : 8 * h + 8, 0:DH],
                                kvall[l][h][
                                    c, :, HKB + tt * H * DH : HKB + (tt + 1) * H * DH
                                ].bitcast(BF16).rearrange(
                                    "p (h e) -> p h e", h=H // 2
                                ),
                            )
                    nc.vector.tensor_copy(
                        out=vcache[:, :, :, 8 * h : 8 * h + 8, DH : DH + 1],
                        in_=padfull_s[:, 0:NDC, :, None, None].to_broadcast(
                            [P, NDC, NT, H // 2, 1]
                        ),
                    )

            def attention(vloc):
                for hp in range(NHP):
                    # packed-layout coordinates of heads (2hp, 2hp+1)
                    sA = 32 * ((2 * hp) % 4)
                    sB = sA + 32
                    f = hp // 2
                    qa = qT8[sA : sA + 32, f]
                    qb = qT8[sB : sB + 32, f]
                    oa = oaccp.tile([P, CH], F32, tag="oacc")
                    ob = oaccp.tile([P, CH], F32, tag="oacc")
                    # (lhsT_a, lhsT_b, vlhsT_a, vlhsT_b, bias, trimul)
                    steps = []
                    for c in range(NDC):
                        for s in range(NT):
                            steps.append(
                                (
                                    kcache[sA : sA + 32, c, f, :, ts(s, P)],
                                    kcache[sB : sB + 32, c, f, :, ts(s, P)],
                                    vcache[:, c, s, 2 * hp],
                                    vcache[:, c, s, 2 * hp + 1],
                                    mdense_s[:, c : c + 1],
                                    None,
                                )
                            )
                    # local diagonal steps first: they only need local q/k/v,
                    # so they overlap with the AllGather + cache fill
                    for j in range(NT):
                        steps.insert(
                            j,
                            (
                                kTloc8[sA : sA + 32, f, :, ts(j, P)],
                                kTloc8[sB : sB + 32, f, :, ts(j, P)],
                                vloc[:, j, 2 * hp],
                                vloc[:, j, 2 * hp + 1],
                                0.0,
                                trim_s[:, j],
                            ),
                        )
                    nsteps = len(steps)
                    for i, (ka, kb, va, vb, bias, tmask) in enumerate(steps):
                        sc = scp.tile([P, 2, CH], F32, tag="sc")
                        nc.tensor.matmul(
                            sc[:, 0], lhsT=ka, rhs=qa, start=True, stop=True,
                            perf_mode=DR, tile_position=(sA, 0),
                        )
                        nc.tensor.matmul(
                            sc[:, 1], lhsT=kb, rhs=qb, start=True, stop=True,
                            perf_mode=DR, tile_position=(sB, 0),
                        )
                        at = attnp.tile([P, 2, CH], BF16, tag="attn")
                        nc.scalar.activation(at, sc, AF.Exp, bias=bias, scale=SCALE)
                        if tmask is not None:
                            nc.vector.tensor_mul(out=at[:, 0], in0=at[:, 0], in1=tmask)
                            nc.vector.tensor_mul(out=at[:, 1], in0=at[:, 1], in1=tmask)
                        nc.tensor.matmul(
                            oa[0 : DH + 1],
                            lhsT=va,
                            rhs=at[:, 0],
                            start=(i == 0),
                            stop=(i == nsteps - 1),
                        )
                        nc.tensor.matmul(
                            ob[0 : DH + 1],
                            lhsT=vb,
                            rhs=at[:, 1],
                            start=(i == 0),
                            stop=(i == nsteps - 1),
                        )
                    # drain psum right away (so the next hp's accumulators
                    # don't wait on the normalize chain): raw o to SBUF +
                    # denominator rows (psum partition 64) staged for recip
                    ls = lrecf.tile([DH + 1, 2, CH], F32, tag="lrecf")
                    nc.vector.tensor_copy(out=ls[64:65, 0], in_=oa[64:65])
                    nc.vector.tensor_copy(out=ls[64:65, 1], in_=ob[64:65])
                    nc.vector.tensor_copy(out=oT[0:64, hp], in_=oa[0:64])
                    nc.vector.tensor_copy(out=oTb[:, hp], in_=ob[0:64])
                    # denominator + oT-half moves ride the ScalarE ring
                    # (idle during attention) so the sync ring is free to
                    # stream out-proj/FFN weights through the whole phase
                    la = lrecf.tile([1, 2, CH], F32, tag="lrecf2")
                    nc.scalar.dma_start(la, ls[64:65])
                    r32 = lrecp.tile([1, 2, CH], F32, tag="lrec")
                    nc.vector.reciprocal_approx_fast(out=r32, in_=la)
                    r16 = lrecp.tile([1, 2, CH], BF16, tag="lrec")
                    nc.vector.tensor_copy(out=r16, in_=r32)
                    nc.gpsimd.partition_broadcast(RbA[:, hp], r16[:, 0])
                    nc.gpsimd.partition_broadcast(RbB[:, hp], r16[:, 1])
                    # normalize this head pair in place right away so
                    # out-proj can consume oT[:, hp] without a global barrier
                    nc.vector.tensor_mul(
                        out=oT[0:64, hp], in0=oT[0:64, hp], in1=RbA[:, hp]
                    )
                    nc.vector.tensor_mul(
                        out=oTb[:, hp], in0=oTb[:, hp], in1=RbB[:, hp]
                    )
                    nc.scalar.dma_start(oT[64:P, hp], oTb[:, hp])

            def outproj(l):
                # m-outer with both weight halves resident: xs[:, m] is
                # final after each m, so the next LN's stats overlap with
                # the remaining m tiles
                wts = []
                for n in range(2):
                    wot = wA.tile([P, NG, CH], BF16, tag="wA")
                    nc.sync.dma_start(
                        wot,
                        wo[l, :, ts(n, CH)].rearrange("(g p) f -> p g f", p=P),
                    )
                    wts.append(wot)
                for m in range(NT):
                    ps = scp.tile([P, 2, CH], F32, tag="sc")
                    for g in range(NG):
                        for n in range(2):
                            nc.tensor.matmul(
                                ps[:, n],
                                lhsT=oT[:, g, ts(m, P)],
                                rhs=wts[n][:, g],
                                start=(g == 0),
                                stop=(g == NG - 1),
                            )
                    psf = ps.rearrange("p n c -> p (n c)")
                    if with_bias:
                        nc.vector.tensor_tensor(
                            psf, psf, bias_bc[:, 1], mybir.AluOpType.add
                        )
                    # residual add + free LN-sum; Square fills ssq overlapped
                    nc.vector.scalar_tensor_tensor(
                        out=xs[:, m], in0=psf, scalar=1.0, in1=xs[:, m],
                        op0=MUL, op1=ADD, accum_out=ssum[:, m : m + 1],
                    )
                    scr = htmp.tile([P, D], BF16, tag="htm")
                    nc.scalar.activation(
                        scr, xs[:, m], AF.Square, accum_out=ssq[:, m : m + 1]
                    )

            def ffn(l, gts):
                for fs in range(NF):
                    w1t = wA.tile([P, NG, P], BF16, tag="wA")
                    nc.sync.dma_start(
                        w1t,
                        w1[l, :, ts(fs, P)].rearrange("(kd p) f -> p kd f", p=P),
                    )
                    ps = mmp.tile([P, CH], F32, tag="mm")
                    for kd in range(NG):
                        nc.tensor.matmul(
                            ps,
                            lhsT=w1t[:, kd],
                            rhs=hT[:, kd],
                            start=(kd == 0),
                            stop=(kd == NG - 1),
                        )
                    nc.scalar.activation(
                        gts[fs // NG][:, fs % NG],
                        ps,
                        AF.Gelu,
                        bias=b1c_s[:, l, fs : fs + 1],
                    )
                for n in range(2):
                    psA = scp.tile([P, 2, CH], F32, tag="sc")
                    psB = scp.tile([P, 2, CH], F32, tag="sc")
                    pslices = [psA[:, 0], psA[:, 1], psB[:, 0], psB[:, 1]]
                    for fs in range(NF):
                        w2t = wB.tile([P, CH], BF16, tag="wB")
                        nc.sync.dma_start(w2t, w2[l, ts(fs, P), ts(n, CH)])
                        for m in range(NT):
                            nc.tensor.matmul(
                                pslices[m],
                                lhsT=gts[fs // NG][:, fs % NG, ts(m, P)],
                                rhs=w2t,
                                start=(fs == 0),
                                stop=(fs == NF - 1),
                            )
                    for m in range(NT):
                        if with_bias:
                            nc.vector.tensor_tensor(
                                pslices[m],
                                pslices[m],
                                bias_bc[:, 2, ts(n, CH)],
                                mybir.AluOpType.add,
                            )
                        # residual add + per-half LN-sum partial; Square on
                        # the finished half fills the ssq partial overlapped
                        nc.vector.scalar_tensor_tensor(
                            out=xs[:, m, ts(n, CH)],
                            in0=pslices[m],
                            scalar=1.0,
                            in1=xs[:, m, ts(n, CH)],
                            op0=MUL,
                            op1=ADD,
                            accum_out=stp[:, n, m : m + 1],
                        )
                        scr = htmp.tile([P, CH], BF16, tag="htm")
                        nc.scalar.activation(
                            scr, xs[:, m, ts(n, CH)], AF.Square,
                            accum_out=stp[:, 2 + n, m : m + 1],
                        )
                # combine the per-half partials into ssum/ssq
                nc.vector.tensor_add(out=ssum, in0=stp[:, 0], in1=stp[:, 1])
                nc.vector.tensor_add(out=ssq, in0=stp[:, 2], in1=stp[:, 3])

            for l in range(nlayers):
                if with_bias:
                    nc.gpsimd.dma_start(
                        bias_bc,
                        brows[None, :, :].to_broadcast([P, 3, D]),
                    )
                vloc = big.tile([P, NT, H, DH + 1], BF16, tag="big")
                ln_to_hT(l, stats_ready=(l > 0))
                # compute exactly the K/V half each collective needs, then
                # trigger it: AG_A (heads 0-7) launches halfway through the
                # QKV phase and AG_B right after, maximizing overlap with
                # Q + the local diagonal attention pass. Packs + kvin writes
                # ride the ScalarE DGE ring (ordered by natural data
                # readiness) so their waits don't stall weight loads.
                qk_one(l, wk, kTloc8, nc.scalar, gs=range(0, NG // 2))
                v_part(l, vloc, ns=[0])
                allgather(l, vloc, hs=[0])
                qk_one(l, wk, kTloc8, nc.scalar, gs=range(NG // 2, NG))
                v_part(l, vloc, ns=[1])
                allgather(l, vloc, hs=[1])
                qk_one(l, wq, qT8, nc.scalar)
                attention(vloc)
                outproj(l)
                ln_to_hT(l, stats_ready=True)
                gts = [
                    big.tile([P, NG, CH], BF16, tag="big", name=f"gts{i}")
                    for i in range(4)
                ]
                ffn(l, gts)

            nc.sync.dma_start(y.rearrange("(tt p) d -> p tt d", p=P), xs)

    nc.finalize()
    return nc


# ------------------------- host side -------------------------


def _layer_norm_np(x):
    mu = x.mean(-1, keepdims=True)
    var = ((x - mu) ** 2).mean(-1, keepdims=True)
    return (x - mu) / np.sqrt(var + 1e-5)


def _sinusoidal_pe():
    pos = np.arange(T, dtype=np.float32)[:, None]
    div = np.exp(np.arange(0, D, 2, dtype=np.float32) * (-np.log(10000.0) / D))
    pe = np.zeros((T, D), dtype=np.float32)
    pe[:, 0::2] = np.sin(pos * div)
    pe[:, 1::2] = np.cos(pos * div)
    return pe


def kernel(**inputs) -> np.ndarray:
    x = np.asarray(inputs["x"], np.float32)
    lengths = np.asarray(inputs["lengths"]).astype(np.int64)
    Wqkv = np.asarray(inputs["Wqkv"], np.float32)
    bqkv = np.asarray(inputs["bqkv"], np.float32)
    Wo = np.asarray(inputs["Wo"], np.float32)
    bo = np.asarray(inputs["bo"], np.float32)
    ln0_g = np.asarray(inputs["ln0_g"], np.float32)
    ln0_b = np.asarray(inputs["ln0_b"], np.float32)
    ln1_g = np.asarray(inputs["ln1_g"], np.float32)
    ln1_b = np.asarray(inputs["ln1_b"], np.float32)
    ln2_g = np.asarray(inputs["ln2_g"], np.float32)
    ln2_b = np.asarray(inputs["ln2_b"], np.float32)
    W1 = np.asarray(inputs["W1"], np.float32)
    b1 = np.asarray(inputs["b1"], np.float32)
    W2 = np.asarray(inputs["W2"], np.float32)
    b2 = np.asarray(inputs["b2"], np.float32)

    bf16 = ml_dtypes.bfloat16
    f8 = ml_dtypes.float8_e4m3

    # LN0 + positional encoding on host
    x0 = _layer_norm_np(x) * ln0_g + ln0_b + _sinusoidal_pe()[None]
    x0 = x0.astype(np.float32)

    # fold ln1/ln2 affine into the first matmul of each block
    Wqkv_eff = ln1_g[:, :, None] * Wqkv
    bqkv_eff = bqkv + np.einsum("ld,ldn->ln", ln1_b, Wqkv)
    W1_eff = ln2_g[:, :, None] * W1
    b1_eff = b1 + np.einsum("ld,ldn->ln", ln2_b, W1)

    bq_eff = bqkv_eff[:, 0:D]
    bk_eff = bqkv_eff[:, D : 2 * D]
    bv_eff = bqkv_eff[:, 2 * D : 3 * D]
    if np.any(bq_eff != 0.0) or np.any(bk_eff != 0.0):
        raise NotImplementedError("nonzero q/k biases not supported")

    # Q/K weights: pre-scale by WS into e4m3 (descaled at the psum copy)
    wq_h = np.ascontiguousarray(Wqkv_eff[:, :, 0:D] * WS).astype(f8)
    wk_h = np.ascontiguousarray(Wqkv_eff[:, :, D : 2 * D] * WS).astype(f8)
    wv_h = np.ascontiguousarray(Wqkv_eff[:, :, 2 * D : 3 * D]).astype(bf16)
    wo_h = Wo.astype(bf16)
    w1_h = W1_eff.astype(bf16)
    w2_h = W2.astype(bf16)

    b1c_h = np.zeros((L, P, NF), np.float32)
    for fs in range(NF):
        b1c_h[:, :, fs] = b1_eff[:, fs * P : (fs + 1) * P]

    with_bias = not (
        np.all(bv_eff == 0.0) and np.all(bo == 0.0) and np.all(b2 == 0.0)
    )
    brows_h = np.stack([bv_eff.sum(0) * 0, bo.sum(0) * 0, b2.sum(0) * 0]).astype(bf16)
    if with_bias:
        # biases are per-layer; the kernel adds the same row each layer, so the
        # general path is only valid when rows are layer-independent.
        same = (
            np.all(bv_eff == bv_eff[0]) and np.all(bo == bo[0]) and np.all(b2 == b2[0])
        )
        if not same:
            raise NotImplementedError("per-layer V/O/FFN2 biases not supported")
        brows_h = np.stack([bv_eff[0], bo[0], b2[0]]).astype(bf16)

    # static triangular masks per diagonal j-tile: [NT, P, CH]
    trim_h = np.zeros((NT, P, CH), np.float32)
    for j in range(NT):
        for i in range(NT):
            blk = trim_h[j][:, i * P : (i + 1) * P]
            if j < i:
                blk[:] = 1.0
            elif j == i:
                blk[:] = np.tril(np.ones((P, P), np.float32)).T  # keep tk <= tq
    trim_h = trim_h.astype(bf16)
    ident_h = np.eye(P, dtype=np.float32).astype(bf16)

    key = with_bias
    if key not in _CACHE:
        _CACHE[key] = _build(with_bias)
    nc = _CACHE[key]

    in_maps = []
    for core in range(8):
        b, q = core // 4, core % 4
        pos = q * CH + np.arange(CH)
        valid = (pos < lengths[b]).astype(np.float32)  # [CH]
        padcol_h = valid.reshape(NT, P).T.copy()  # [P, NT]
        posf = np.arange(T)
        validf = (posf < lengths[b]).astype(np.float32)
        padfull_h = validf.reshape(NCHUNK, NT, P).transpose(2, 0, 1).copy()
        mdense_h = np.zeros((P, NCHUNK), np.float32)
        for c in range(NCHUNK):
            if c >= q:
                mdense_h[:, c] = NEG
        m = {
            "x0": np.ascontiguousarray(x0[b, q * CH : (q + 1) * CH]),
            "wq": wq_h,
            "wk": wk_h,
            "wv": wv_h,
            "wo": wo_h,
            "w1": w1_h,
            "w2": w2_h,
            "b1c": b1c_h,
            "mdense": mdense_h,
            "padcol": padcol_h,
            "padfull": padfull_h,
            "trim": np.ascontiguousarray(trim_h),
            "ident": ident_h,
        }
        if with_bias:
            m["brows"] = brows_h
        in_maps.append(m)

    res = run_bass_kernel_spmd(
        nc,
        in_maps,
        core_ids=list(range(8)),
        trace=bool(os.environ.get("KERNEL_TRACE")),
    )
    globals()["LAST_RESULT"] = res
    out = np.zeros((B, T, D), np.float32)
    for core in range(8):
        b, q = core // 4, core % 4
        out[b, q * CH : (q + 1) * CH] = res.results[core]["y"]
    return out
